# revision 27
# baseline (speedup 1.0000x reference)
"""Causal multi-head attention (B=4, T=2048, D=1024, 16 heads x 64) on 8 trn2 cores.

Sharding: tensor-parallel over heads, 2 heads per core. Every core receives the
full activations x (pre-transposed on host to [B, D, T], cast bf16) plus its 2
heads' worth of W_Q/W_K/W_V pre-arranged to [128, 8*128] bf16; it computes full
causal attention for its heads and writes out z^T plus the softmax denominator
row per head ([B, 2, 65, T] f32). The host normalizes and re-lays-out.

Device kernel layout choices (per core, HW ~266 us):
  - projections produce Q^T/K^T/V^T head-major [128(2h), T]; V^T is
    PE-transposed per 128-block into the AV stationary (v_aug).
  - scores computed transposed S^T[kt, qt] so the two heads run as concurrent
    PE row-tiles (K=64 at partition bases 0/64, ~4ns apart) into the two
    halves of one [128, 1024] PSUM pair; one Exp (scale=1/8 fused) per chunk.
  - causal mask applied post-exp with gpsimd affine_select (idle engine);
    diagonal chunks narrowed to their valid column range.
  - softmax denominator comes free from an all-ones column in v_aug (padded
    to 128 columns for fast weight load), accumulated in the same f32 PSUM
    as z^T; AV is software-pipelined one chunk behind the scores.
  - all matmuls bf16 with fp32 PSUM accumulation (end-to-end rel err ~5e-3);
    set PROJ_F32R=True for fp32r projections (~3.5e-3, ~13% slower).
"""

import os
import sys

for _p in ("/opt/trn_rl_repo", "/root/.axon_site/_ro/trn_rl_repo"):
    if os.path.isdir(_p) and _p not in sys.path:
        sys.path.insert(0, _p)

import ml_dtypes
import numpy as np

import concourse.bass as bass
import concourse.mybir as mybir
import concourse.tile as tile
from concourse import bacc
from concourse.bass import ds
from concourse.bass_utils import run_bass_kernel_spmd
from concourse.masks import make_identity

B, T, D = 4, 2048, 1024
NH, DH = 16, 64
NCORES = 8
HPC = NH // NCORES          # heads per core = 2
H2 = HPC * DH               # packed per-core head dim = 128
P = 128
QT = 512                    # query-tile width (psum bank limit for f32 out)
NQ4 = T // QT               # 4 query tiles
NCH = T // P                # 16 key chunks
KD = D // P                 # 8 contraction chunks
F32 = mybir.dt.float32
BF16 = mybir.dt.bfloat16
F32R = mybir.dt.float32r
PROJ_F32R = False  # fp32r projections: better precision, slightly slower
SCALE = 1.0 / np.sqrt(DH)   # 0.125


def _build(nc, tc, xT_d, w_d, cst_d, out_d):
    from contextlib import ExitStack

    AF = mybir.ActivationFunctionType
    OP = mybir.AluOpType
    MPB = QT // P  # 128-blocks per query tile = 4

    with ExitStack() as ctx:
        ep = ctx.enter_context
        const = ep(tc.tile_pool(name="const", bufs=1))
        xt_pool = ep(tc.tile_pool(name="xt", bufs=2 * KD + 1))
        qk_pool = ep(tc.tile_pool(name="qk", bufs=2))
        vt_pool = ep(tc.tile_pool(name="vt", bufs=3))
        vaug_pool = ep(tc.tile_pool(name="vaug", bufs=2))
        p_pool = ep(tc.tile_pool(name="pp", bufs=8))
        zt_pool = ep(tc.tile_pool(name="zt", bufs=2))
        ps_acc = ep(tc.tile_pool(name="ps_acc", bufs=2, space="PSUM"))
        ps_s = ep(tc.tile_pool(name="ps_s", bufs=2, space="PSUM"))
        ps_z = ep(tc.tile_pool(name="ps_z", bufs=2, space="PSUM"))

        # DMA order matters at startup: first x chunk, then the weights the
        # first matmul group needs, then the rest of batch 0's x.
        XDT = F32R if PROJ_F32R else BF16
        xch0 = [xt_pool.tile([P, T], XDT, tag="xt", name="xt_t") for _ in range(KD)]
        nc.sync.dma_start(xch0[0][:], xT_d[0, ds(0, P), :])
        w_sb = {}
        for name in ("wq", "wk", "wv"):
            t = const.tile([P, KD, H2], XDT, tag=name)
            nc.sync.dma_start(t[:], w_d[name].rearrange("p (c h) -> p c h", c=KD))
            w_sb[name] = t
        for k in range(1, KD):
            nc.sync.dma_start(xch0[k][:], xT_d[0, ds(k * P, P), :])

        ident = const.tile([P, P], BF16, tag="ident")
        make_identity(nc, ident)

        # v_aug double buffers: [kt, chunk, 64 v-cols | ones col | 63 zeros]
        # (padded to 128 columns so the AV weight load can use FWL)
        vaug = []
        for _bb in range(2):
            pair = []
            for h in range(HPC):
                v = vaug_pool.tile([P, NCH, P], BF16, tag=f"v{h}")
                nc.gpsimd.memset(v[:, :, DH:P], 0.0)
                nc.gpsimd.memset(v[:, :, DH : DH + 1], 1.0)
                pair.append(v)
            vaug.append(pair)

        for b in range(B):
            # ---- stream x^T chunks [128d, T] ----
            if b == 0:
                xch = xch0
            else:
                xch = []
                for k in range(KD):
                    xt_t = xt_pool.tile([P, T], XDT, tag="xt", name="xt_t")
                    nc.sync.dma_start(xt_t[:], xT_d[b, ds(k * P, P), :])
                    xch.append(xt_t)

            # ---- projections: Q^T, K^T (kept), V^T (transposed to v_aug) ----
            qt_sb = qk_pool.tile([P, T], BF16, tag="qt")
            kt_sb = qk_pool.tile([P, T], BF16, tag="kt")
            va = vaug[b % 2]
            for t4 in range(NQ4):
                for name, dst in (("wq", qt_sb), ("wk", kt_sb), ("wv", None)):
                    acc = ps_acc.tile([P, QT], F32, tag="acc")
                    for k in range(KD):
                        nc.tensor.matmul(
                            acc[:],
                            w_sb[name][:, k, :],
                            xch[k][:, ds(t4 * QT, QT)],
                            start=(k == 0),
                            stop=(k == KD - 1),
                        )
                    if dst is not None:
                        nc.vector.tensor_copy(dst[:, ds(t4 * QT, QT)], acc[:])
                    else:
                        vt_t = vt_pool.tile([P, QT], BF16, tag="vt")
                        nc.vector.tensor_copy(vt_t[:], acc[:])
                        for m in range(MPB):
                            j = t4 * MPB + m
                            pt = ps_acc.tile([P, P], BF16, tag="acc", name="pt")
                            nc.tensor.transpose(
                                pt[:], vt_t[:, ds(m * P, P)], ident[:]
                            )
                            for h in range(HPC):
                                nc.vector.tensor_copy(
                                    va[h][:, j, 0:DH], pt[:, ds(h * DH, DH)]
                                )

            # ---- causal attention; scores for both heads side by side ----
            zt_sb = [
                zt_pool.tile([DH + 1, T], F32, tag=f"z{h}", name=f"ztb{h}")
                for h in range(HPC)
            ]
            for q4 in range(NQ4):
                njs = (q4 + 1) * MPB
                pz = [
                    ps_z.tile([P, QT], F32, tag="z", name="pz") for _ in range(HPC)
                ]
                pend = []  # (j, c0, exp tile) awaiting the AV matmuls
                for j in range(njs):
                    rdiag = j - q4 * MPB  # >=0 on diagonal-overlap chunks
                    last = j == njs - 1
                    c0 = 0 if rdiag < 0 else rdiag * P
                    w_hi = (rdiag + 1) * P if rdiag >= 0 else 0
                    nw = QT - c0
                    ss = ps_s.tile([P, 2 * QT], F32, tag="s")
                    pe = p_pool.tile([P, 2 * QT], BF16, tag="p", name="pe")
                    for h in range(HPC):
                        hp = ds(h * DH, DH)
                        nc.tensor.matmul(
                            ss[:, h * QT + c0 : (h + 1) * QT],
                            kt_sb[hp, ds(j * P, P)],
                            qt_sb[hp, ds(q4 * QT + c0, nw)],
                            start=True,
                            stop=True,
                        )
                    # one exp covering both heads' valid halves
                    if c0 == 0:
                        nc.scalar.activation(
                            pe[:, :], ss[:, :], AF.Exp, scale=float(SCALE)
                        )
                    else:
                        for h in range(HPC):
                            nc.scalar.activation(
                                pe[:, h * QT + c0 : (h + 1) * QT],
                                ss[:, h * QT + c0 : (h + 1) * QT],
                                AF.Exp,
                                scale=float(SCALE),
                            )
                    if rdiag >= 0:
                        # keep iff qt >= kt  <=>  (col - p - 128*rdiag) >= 0
                        for h in range(HPC):
                            nc.gpsimd.affine_select(
                                out=pe[:, h * QT + c0 : h * QT + w_hi],
                                in_=pe[:, h * QT + c0 : h * QT + w_hi],
                                compare_op=OP.is_ge,
                                fill=0.0,
                                base=c0 - rdiag * P,
                                pattern=[[1, w_hi - c0]],
                                channel_multiplier=-1,
                            )
                    pend.append((j, c0, pe))
                    # software-pipeline: AV runs one chunk behind the scores
                    if len(pend) > 1 or last:
                        for jj, cc0, ppe in pend if last else [pend[0]]:
                            for h in range(HPC):
                                nc.tensor.matmul(
                                    pz[h][:, cc0:QT],
                                    va[h][:, jj, :],
                                    ppe[:, h * QT + cc0 : (h + 1) * QT],
                                    start=(jj == 0),
                                    stop=(jj == njs - 1),
                                    skip_group_check=True,
                                )
                        pend = [] if last else pend[1:]

                for h in range(HPC):
                    nc.vector.tensor_copy(
                        zt_sb[h][:, ds(q4 * QT, QT)], pz[h][0 : DH + 1, :]
                    )
                    # z^T (+ denominator row) raw; host divides and transposes
                    nc.sync.dma_start(
                        out_d[b, h, :, ds(q4 * QT, QT)],
                        zt_sb[h][:, ds(q4 * QT, QT)],
                    )


def build_bass():
    nc = bacc.Bacc(None, target_bir_lowering=False)
    xT_d = nc.declare_dram_parameter(
        "xT", [B, D, T], F32R if PROJ_F32R else BF16, isOutput=False
    )
    w_d = {
        name: nc.declare_dram_parameter(
            name, [P, KD * H2], F32R if PROJ_F32R else BF16, isOutput=False
        )
        for name in ("wq", "wk", "wv")
    }
    cst_d = {}
    out_d = nc.declare_dram_parameter(
        "out", [B, HPC, DH + 1, T], F32, isOutput=True
    )
    with tile.TileContext(nc) as tc:
        _build(nc, tc, xT_d, w_d, cst_d, out_d)
    nc.compile()
    return nc


_CACHE = {}


def _get_nc():
    if "nc" not in _CACHE:
        _CACHE["nc"] = build_bass()
    return _CACHE["nc"]


def make_in_maps(x, W_K, W_Q, W_V):
    x = np.asarray(x, dtype=np.float32)
    xT = np.ascontiguousarray(np.transpose(x, (0, 2, 1)))
    if not PROJ_F32R:
        xT = xT.astype(ml_dtypes.bfloat16)
    in_maps = []
    for c in range(NCORES):
        sl = slice(c * HPC, (c + 1) * HPC)

        def wt(w):
            w = np.asarray(w, dtype=np.float32)
            wt_ = w[sl].reshape(H2, D).T  # [D, H2]
            wt_ = wt_.reshape(KD, P, H2).transpose(1, 0, 2).reshape(P, KD * H2)
            wt_ = np.ascontiguousarray(wt_)
            return wt_ if PROJ_F32R else wt_.astype(ml_dtypes.bfloat16)

        in_maps.append({"xT": xT, "wq": wt(W_Q), "wk": wt(W_K), "wv": wt(W_V)})
    return in_maps


def kernel(x, W_K, W_Q, W_V, _trace=False, _trace_kwargs=None):
    in_maps = make_in_maps(x, W_K, W_Q, W_V)
    res = run_bass_kernel_spmd(
        _get_nc(),
        in_maps,
        list(range(NCORES)),
        trace=_trace,
        **(_trace_kwargs or {}),
    )
    _CACHE["last_results"] = res
    outs = []
    for c in range(NCORES):
        zt = np.asarray(res.results[c]["out"])  # [B, HPC, DH+1, T]
        z = zt[:, :, :DH, :] / zt[:, :, DH : DH + 1, :]
        outs.append(np.transpose(z, (0, 3, 1, 2)).reshape(B, T, H2))
    return np.concatenate(outs, axis=2)


# revision 29
# speedup vs baseline: 1.0352x; 1.0352x over previous
"""Causal multi-head attention (B=4, T=2048, D=1024, 16 heads x 64) on 8 trn2 cores.

Sharding: tensor-parallel over heads, 2 heads per core. Every core receives the
full activations x (pre-transposed on host to [B, D, T], cast bf16) plus its 2
heads' worth of W_Q/W_K/W_V pre-arranged to [128, 8*128] bf16; it computes full
causal attention for its heads and writes out z^T plus the softmax denominator
row per head ([B, 2, 65, T] f32). The host normalizes and re-lays-out.

Device kernel layout choices (per core, HW ~266 us):
  - projections produce Q^T/K^T/V^T head-major [128(2h), T]; V^T is
    PE-transposed per 128-block into the AV stationary (v_aug).
  - scores computed transposed S^T[kt, qt] so the two heads run as concurrent
    PE row-tiles (K=64 at partition bases 0/64, ~4ns apart) into the two
    halves of one [128, 1024] PSUM pair; one Exp (scale=1/8 fused) per chunk.
  - causal mask applied post-exp with gpsimd affine_select (idle engine);
    diagonal chunks narrowed to their valid column range.
  - softmax denominator comes free from an all-ones column in v_aug (padded
    to 128 columns for fast weight load), accumulated in the same f32 PSUM
    as z^T; AV is software-pipelined one chunk behind the scores.
  - all matmuls bf16 with fp32 PSUM accumulation (end-to-end rel err ~5e-3);
    set PROJ_F32R=True for fp32r projections (~3.5e-3, ~13% slower).
"""

import os
import sys

for _p in ("/opt/trn_rl_repo", "/root/.axon_site/_ro/trn_rl_repo"):
    if os.path.isdir(_p) and _p not in sys.path:
        sys.path.insert(0, _p)

import ml_dtypes
import numpy as np

import concourse.bass as bass
import concourse.mybir as mybir
import concourse.tile as tile
from concourse import bacc
from concourse.bass import ds
from concourse.bass_utils import run_bass_kernel_spmd
from concourse.masks import make_identity

B, T, D = 4, 2048, 1024
NH, DH = 16, 64
NCORES = 8
HPC = NH // NCORES          # heads per core = 2
H2 = HPC * DH               # packed per-core head dim = 128
P = 128
QT = 512                    # query-tile width (psum bank limit for f32 out)
NQ4 = T // QT               # 4 query tiles
NCH = T // P                # 16 key chunks
KD = D // P                 # 8 contraction chunks
F32 = mybir.dt.float32
BF16 = mybir.dt.bfloat16
F32R = mybir.dt.float32r
PROJ_F32R = False  # fp32r projections: better precision, slightly slower
SCALE = 1.0 / np.sqrt(DH)   # 0.125


def _build(nc, tc, xT_d, w_d, cst_d, out_d):
    from contextlib import ExitStack

    AF = mybir.ActivationFunctionType
    OP = mybir.AluOpType
    MPB = QT // P  # 128-blocks per query tile = 4

    with ExitStack() as ctx:
        ep = ctx.enter_context
        const = ep(tc.tile_pool(name="const", bufs=1))
        xt_pool = ep(tc.tile_pool(name="xt", bufs=2 * KD + 1))
        qk_pool = ep(tc.tile_pool(name="qk", bufs=2))
        vt_pool = ep(tc.tile_pool(name="vt", bufs=3))
        vaug_pool = ep(tc.tile_pool(name="vaug", bufs=2))
        p_pool = ep(tc.tile_pool(name="pp", bufs=8))
        zt_pool = ep(tc.tile_pool(name="zt", bufs=2))
        ps_acc = ep(tc.tile_pool(name="ps_acc", bufs=2, space="PSUM"))
        ps_s = ep(tc.tile_pool(name="ps_s", bufs=2, space="PSUM"))
        ps_z = ep(tc.tile_pool(name="ps_z", bufs=2, space="PSUM"))

        # DMA order matters at startup: first x chunk, then the weights the
        # first matmul group needs, then the rest of batch 0's x.
        XDT = F32R if PROJ_F32R else BF16
        xch0 = [xt_pool.tile([P, T], XDT, tag="xt", name="xt_t") for _ in range(KD)]
        nc.sync.dma_start(xch0[0][:], xT_d[0, ds(0, P), :])
        w_sb = {}
        for name in ("wq", "wk", "wv"):
            t = const.tile([P, KD, H2], XDT, tag=name)
            nc.sync.dma_start(t[:], w_d[name].rearrange("p (c h) -> p c h", c=KD))
            w_sb[name] = t
        for k in range(1, KD):
            nc.sync.dma_start(xch0[k][:], xT_d[0, ds(k * P, P), :])

        ident = const.tile([P, P], BF16, tag="ident")
        make_identity(nc, ident)


        # v_aug double buffers: [kt, chunk, 64 v-cols | ones col | 63 zeros]
        # (padded to 128 columns so the AV weight load can use FWL)
        vaug = []
        for _bb in range(2):
            pair = []
            for h in range(HPC):
                v = vaug_pool.tile([P, NCH, P], BF16, tag=f"v{h}")
                nc.gpsimd.memset(v[:, :, DH:P], 0.0)
                nc.gpsimd.memset(v[:, :, DH : DH + 1], 1.0)
                pair.append(v)
            vaug.append(pair)

        for b in range(B):
            # ---- stream x^T chunks [128d, T] ----
            if b == 0:
                xch = xch0
            else:
                xch = []
                for k in range(KD):
                    xt_t = xt_pool.tile([P, T], XDT, tag="xt", name="xt_t")
                    nc.sync.dma_start(xt_t[:], xT_d[b, ds(k * P, P), :])
                    xch.append(xt_t)

            # ---- projections: Q^T, K^T (kept), V^T (transposed to v_aug) ----
            qt_sb = qk_pool.tile([P, T], BF16, tag="qt")
            kt_sb = qk_pool.tile([P, T], BF16, tag="kt")
            va = vaug[b % 2]
            for t4 in range(NQ4):
                for name, dst in (("wq", qt_sb), ("wk", kt_sb), ("wv", None)):
                    acc = ps_acc.tile([P, QT], F32, tag="acc")
                    for k in range(KD):
                        nc.tensor.matmul(
                            acc[:],
                            w_sb[name][:, k, :],
                            xch[k][:, ds(t4 * QT, QT)],
                            start=(k == 0),
                            stop=(k == KD - 1),
                        )
                    if dst is not None:
                        nc.vector.tensor_copy(dst[:, ds(t4 * QT, QT)], acc[:])
                    else:
                        vt_t = vt_pool.tile([P, QT], BF16, tag="vt")
                        nc.vector.tensor_copy(vt_t[:], acc[:])
                        for m in range(MPB):
                            j = t4 * MPB + m
                            pt = ps_acc.tile([P, P], BF16, tag="acc", name="pt")
                            nc.tensor.transpose(
                                pt[:], vt_t[:, ds(m * P, P)], ident[:]
                            )
                            for h in range(HPC):
                                nc.vector.tensor_copy(
                                    va[h][:, j, 0:DH], pt[:, ds(h * DH, DH)]
                                )

            # ---- causal attention; scores for both heads side by side ----
            zt_sb = [
                zt_pool.tile([DH + 1, T], F32, tag=f"z{h}", name=f"ztb{h}")
                for h in range(HPC)
            ]
            for q4 in range(NQ4):
                njs = (q4 + 1) * MPB
                pz = [
                    ps_z.tile([P, QT], F32, tag="z", name="pz") for _ in range(HPC)
                ]
                pend = []  # (j, c0, exp tile) awaiting the AV matmuls
                for j in range(njs):
                    rdiag = j - q4 * MPB  # >=0 on diagonal-overlap chunks
                    last = j == njs - 1
                    c0 = 0 if rdiag < 0 else rdiag * P
                    w_hi = (rdiag + 1) * P if rdiag >= 0 else 0
                    nw = QT - c0
                    ss = ps_s.tile([P, 2 * QT], F32, tag="s")
                    pe = p_pool.tile([P, 2 * QT], BF16, tag="p", name="pe")
                    for h in range(HPC):
                        hp = ds(h * DH, DH)
                        nc.tensor.matmul(
                            ss[:, h * QT + c0 : (h + 1) * QT],
                            kt_sb[hp, ds(j * P, P)],
                            qt_sb[hp, ds(q4 * QT + c0, nw)],
                            start=True,
                            stop=True,
                        )
                    # one exp covering both heads' valid halves
                    if c0 == 0:
                        nc.scalar.activation(
                            pe[:, :], ss[:, :], AF.Exp, scale=float(SCALE)
                        )
                    else:
                        for h in range(HPC):
                            nc.scalar.activation(
                                pe[:, h * QT + c0 : (h + 1) * QT],
                                ss[:, h * QT + c0 : (h + 1) * QT],
                                AF.Exp,
                                scale=float(SCALE),
                            )
                    if rdiag >= 0:
                        # keep iff qt >= kt  <=>  (col - p - 128*rdiag) >= 0
                        for h in range(HPC):
                            nc.gpsimd.affine_select(
                                out=pe[:, h * QT + c0 : h * QT + w_hi],
                                in_=pe[:, h * QT + c0 : h * QT + w_hi],
                                compare_op=OP.is_ge,
                                fill=0.0,
                                base=c0 - rdiag * P,
                                pattern=[[1, w_hi - c0]],
                                channel_multiplier=-1,
                            )
                    pend.append((j, c0, pe))
                    # software-pipeline: AV runs one chunk behind the scores
                    if len(pend) > 2 or last:
                        for jj, cc0, ppe in pend if last else [pend[0]]:
                            for h in range(HPC):
                                nc.tensor.matmul(
                                    pz[h][:, cc0:QT],
                                    va[h][:, jj, :],
                                    ppe[:, h * QT + cc0 : (h + 1) * QT],
                                    start=(jj == 0),
                                    stop=(jj == njs - 1),
                                    skip_group_check=True,
                                )
                        pend = [] if last else pend[1:]

                for h in range(HPC):
                    nc.vector.tensor_copy(
                        zt_sb[h][:, ds(q4 * QT, QT)], pz[h][0 : DH + 1, :]
                    )
                    # z^T (+ denominator row) raw; host divides and transposes
                    nc.sync.dma_start(
                        out_d[b, h, :, ds(q4 * QT, QT)],
                        zt_sb[h][:, ds(q4 * QT, QT)],
                    )


def build_bass():
    nc = bacc.Bacc(None, target_bir_lowering=False)
    xT_d = nc.declare_dram_parameter(
        "xT", [B, D, T], F32R if PROJ_F32R else BF16, isOutput=False
    )
    w_d = {
        name: nc.declare_dram_parameter(
            name, [P, KD * H2], F32R if PROJ_F32R else BF16, isOutput=False
        )
        for name in ("wq", "wk", "wv")
    }
    cst_d = {}
    out_d = nc.declare_dram_parameter(
        "out", [B, HPC, DH + 1, T], F32, isOutput=True
    )
    with tile.TileContext(nc) as tc:
        _build(nc, tc, xT_d, w_d, cst_d, out_d)
    nc.compile()
    return nc


_CACHE = {}


def _get_nc():
    if "nc" not in _CACHE:
        _CACHE["nc"] = build_bass()
    return _CACHE["nc"]


def make_in_maps(x, W_K, W_Q, W_V):
    x = np.asarray(x, dtype=np.float32)
    xT = np.ascontiguousarray(np.transpose(x, (0, 2, 1)))
    if not PROJ_F32R:
        xT = xT.astype(ml_dtypes.bfloat16)
    in_maps = []
    for c in range(NCORES):
        sl = slice(c * HPC, (c + 1) * HPC)

        def wt(w):
            w = np.asarray(w, dtype=np.float32)
            wt_ = w[sl].reshape(H2, D).T  # [D, H2]
            wt_ = wt_.reshape(KD, P, H2).transpose(1, 0, 2).reshape(P, KD * H2)
            wt_ = np.ascontiguousarray(wt_)
            return wt_ if PROJ_F32R else wt_.astype(ml_dtypes.bfloat16)

        in_maps.append({"xT": xT, "wq": wt(W_Q), "wk": wt(W_K), "wv": wt(W_V)})
    return in_maps


def kernel(x, W_K, W_Q, W_V, _trace=False, _trace_kwargs=None):
    in_maps = make_in_maps(x, W_K, W_Q, W_V)
    res = run_bass_kernel_spmd(
        _get_nc(),
        in_maps,
        list(range(NCORES)),
        trace=_trace,
        **(_trace_kwargs or {}),
    )
    _CACHE["last_results"] = res
    outs = []
    for c in range(NCORES):
        zt = np.asarray(res.results[c]["out"])  # [B, HPC, DH+1, T]
        z = zt[:, :, :DH, :] / zt[:, :, DH : DH + 1, :]
        outs.append(np.transpose(z, (0, 3, 1, 2)).reshape(B, T, H2))
    return np.concatenate(outs, axis=2)


# revision 30
# speedup vs baseline: 1.0373x; 1.0020x over previous
"""Causal multi-head attention (B=4, T=2048, D=1024, 16 heads x 64) on 8 trn2 cores.

Sharding: tensor-parallel over heads, 2 heads per core. Every core receives the
full activations x (pre-transposed on host to [B, D, T], cast bf16) plus its 2
heads' worth of W_Q/W_K/W_V pre-arranged to [128, 8*128] bf16; it computes full
causal attention for its heads and writes out z^T plus the softmax denominator
row per head ([B, 2, 65, T] f32). The host normalizes and re-lays-out.

Device kernel layout choices (per core, HW ~266 us):
  - projections produce Q^T/K^T/V^T head-major [128(2h), T]; V^T is
    PE-transposed per 128-block into the AV stationary (v_aug).
  - scores computed transposed S^T[kt, qt] so the two heads run as concurrent
    PE row-tiles (K=64 at partition bases 0/64, ~4ns apart) into the two
    halves of one [128, 1024] PSUM pair; one Exp (scale=1/8 fused) per chunk.
  - causal mask applied post-exp with gpsimd affine_select (idle engine);
    diagonal chunks narrowed to their valid column range.
  - softmax denominator comes free from an all-ones column in v_aug (padded
    to 128 columns for fast weight load), accumulated in the same f32 PSUM
    as z^T; AV is software-pipelined one chunk behind the scores.
  - all matmuls bf16 with fp32 PSUM accumulation (end-to-end rel err ~5e-3);
    set PROJ_F32R=True for fp32r projections (~3.5e-3, ~13% slower).
"""

import os
import sys

for _p in ("/opt/trn_rl_repo", "/root/.axon_site/_ro/trn_rl_repo"):
    if os.path.isdir(_p) and _p not in sys.path:
        sys.path.insert(0, _p)

import ml_dtypes
import numpy as np

import concourse.bass as bass
import concourse.mybir as mybir
import concourse.tile as tile
from concourse import bacc
from concourse.bass import ds
from concourse.bass_utils import run_bass_kernel_spmd
from concourse.masks import make_identity

B, T, D = 4, 2048, 1024
NH, DH = 16, 64
NCORES = 8
HPC = NH // NCORES          # heads per core = 2
H2 = HPC * DH               # packed per-core head dim = 128
P = 128
QT = 512                    # query-tile width (psum bank limit for f32 out)
NQ4 = T // QT               # 4 query tiles
NCH = T // P                # 16 key chunks
KD = D // P                 # 8 contraction chunks
F32 = mybir.dt.float32
BF16 = mybir.dt.bfloat16
F32R = mybir.dt.float32r
PROJ_F32R = False  # fp32r projections: better precision, slightly slower
SCALE = 1.0 / np.sqrt(DH)   # 0.125


def _build(nc, tc, xT_d, w_d, cst_d, out_d):
    from contextlib import ExitStack

    AF = mybir.ActivationFunctionType
    OP = mybir.AluOpType
    MPB = QT // P  # 128-blocks per query tile = 4

    with ExitStack() as ctx:
        ep = ctx.enter_context
        const = ep(tc.tile_pool(name="const", bufs=1))
        xt_pool = ep(tc.tile_pool(name="xt", bufs=2 * KD + 1))
        qk_pool = ep(tc.tile_pool(name="qk", bufs=2))
        vt_pool = ep(tc.tile_pool(name="vt", bufs=3))
        vaug_pool = ep(tc.tile_pool(name="vaug", bufs=2))
        p_pool = ep(tc.tile_pool(name="pp", bufs=10))
        zt_pool = ep(tc.tile_pool(name="zt", bufs=2))
        ps_acc = ep(tc.tile_pool(name="ps_acc", bufs=2, space="PSUM"))
        ps_s = ep(tc.tile_pool(name="ps_s", bufs=2, space="PSUM"))
        ps_z = ep(tc.tile_pool(name="ps_z", bufs=2, space="PSUM"))

        # DMA order matters at startup: first x chunk, then the weights the
        # first matmul group needs, then the rest of batch 0's x.
        XDT = F32R if PROJ_F32R else BF16
        xch0 = [xt_pool.tile([P, T], XDT, tag="xt", name="xt_t") for _ in range(KD)]
        nc.sync.dma_start(xch0[0][:], xT_d[0, ds(0, P), :])
        w_sb = {}
        for name in ("wq", "wk", "wv"):
            t = const.tile([P, KD, H2], XDT, tag=name)
            nc.sync.dma_start(t[:], w_d[name].rearrange("p (c h) -> p c h", c=KD))
            w_sb[name] = t
        for k in range(1, KD):
            nc.sync.dma_start(xch0[k][:], xT_d[0, ds(k * P, P), :])

        ident = const.tile([P, P], BF16, tag="ident")
        make_identity(nc, ident)


        # v_aug double buffers: [kt, chunk, 64 v-cols | ones col | 63 zeros]
        # (padded to 128 columns so the AV weight load can use FWL)
        vaug = []
        for _bb in range(2):
            pair = []
            for h in range(HPC):
                v = vaug_pool.tile([P, NCH, P], BF16, tag=f"v{h}")
                nc.gpsimd.memset(v[:, :, DH:P], 0.0)
                nc.gpsimd.memset(v[:, :, DH : DH + 1], 1.0)
                pair.append(v)
            vaug.append(pair)

        for b in range(B):
            # ---- stream x^T chunks [128d, T] ----
            if b == 0:
                xch = xch0
            else:
                xch = []
                for k in range(KD):
                    xt_t = xt_pool.tile([P, T], XDT, tag="xt", name="xt_t")
                    nc.sync.dma_start(xt_t[:], xT_d[b, ds(k * P, P), :])
                    xch.append(xt_t)

            # ---- projections: Q^T, K^T (kept), V^T (transposed to v_aug) ----
            qt_sb = qk_pool.tile([P, T], BF16, tag="qt")
            kt_sb = qk_pool.tile([P, T], BF16, tag="kt")
            va = vaug[b % 2]
            for t4 in range(NQ4):
                for name, dst in (("wq", qt_sb), ("wk", kt_sb), ("wv", None)):
                    acc = ps_acc.tile([P, QT], F32, tag="acc")
                    for k in range(KD):
                        nc.tensor.matmul(
                            acc[:],
                            w_sb[name][:, k, :],
                            xch[k][:, ds(t4 * QT, QT)],
                            start=(k == 0),
                            stop=(k == KD - 1),
                        )
                    if dst is not None:
                        nc.vector.tensor_copy(dst[:, ds(t4 * QT, QT)], acc[:])
                    else:
                        vt_t = vt_pool.tile([P, QT], BF16, tag="vt")
                        nc.vector.tensor_copy(vt_t[:], acc[:])
                        for m in range(MPB):
                            j = t4 * MPB + m
                            pt = ps_acc.tile([P, P], BF16, tag="acc", name="pt")
                            nc.tensor.transpose(
                                pt[:], vt_t[:, ds(m * P, P)], ident[:]
                            )
                            for h in range(HPC):
                                nc.vector.tensor_copy(
                                    va[h][:, j, 0:DH], pt[:, ds(h * DH, DH)]
                                )

            # ---- causal attention; scores for both heads side by side ----
            zt_sb = [
                zt_pool.tile([DH + 1, T], F32, tag=f"z{h}", name=f"ztb{h}")
                for h in range(HPC)
            ]
            for q4 in range(NQ4):
                njs = (q4 + 1) * MPB
                pz = [
                    ps_z.tile([P, QT], F32, tag="z", name="pz") for _ in range(HPC)
                ]
                pend = []  # (j, c0, exp tile) awaiting the AV matmuls
                for j in range(njs):
                    rdiag = j - q4 * MPB  # >=0 on diagonal-overlap chunks
                    last = j == njs - 1
                    c0 = 0 if rdiag < 0 else rdiag * P
                    w_hi = (rdiag + 1) * P if rdiag >= 0 else 0
                    nw = QT - c0
                    ss = ps_s.tile([P, 2 * QT], F32, tag="s")
                    pe = p_pool.tile([P, 2 * QT], BF16, tag="p", name="pe")
                    for h in range(HPC):
                        hp = ds(h * DH, DH)
                        nc.tensor.matmul(
                            ss[:, h * QT + c0 : (h + 1) * QT],
                            kt_sb[hp, ds(j * P, P)],
                            qt_sb[hp, ds(q4 * QT + c0, nw)],
                            start=True,
                            stop=True,
                        )
                    # one exp covering both heads' valid halves
                    if c0 == 0:
                        nc.scalar.activation(
                            pe[:, :], ss[:, :], AF.Exp, scale=float(SCALE)
                        )
                    else:
                        for h in range(HPC):
                            nc.scalar.activation(
                                pe[:, h * QT + c0 : (h + 1) * QT],
                                ss[:, h * QT + c0 : (h + 1) * QT],
                                AF.Exp,
                                scale=float(SCALE),
                            )
                    if rdiag >= 0:
                        # keep iff qt >= kt  <=>  (col - p - 128*rdiag) >= 0
                        for h in range(HPC):
                            nc.gpsimd.affine_select(
                                out=pe[:, h * QT + c0 : h * QT + w_hi],
                                in_=pe[:, h * QT + c0 : h * QT + w_hi],
                                compare_op=OP.is_ge,
                                fill=0.0,
                                base=c0 - rdiag * P,
                                pattern=[[1, w_hi - c0]],
                                channel_multiplier=-1,
                            )
                    pend.append((j, c0, pe))
                    # software-pipeline: AV runs one chunk behind the scores
                    if len(pend) > 3 or last:
                        for jj, cc0, ppe in pend if last else [pend[0]]:
                            for h in range(HPC):
                                nc.tensor.matmul(
                                    pz[h][:, cc0:QT],
                                    va[h][:, jj, :],
                                    ppe[:, h * QT + cc0 : (h + 1) * QT],
                                    start=(jj == 0),
                                    stop=(jj == njs - 1),
                                    skip_group_check=True,
                                )
                        pend = [] if last else pend[1:]

                for h in range(HPC):
                    nc.vector.tensor_copy(
                        zt_sb[h][:, ds(q4 * QT, QT)], pz[h][0 : DH + 1, :]
                    )
                    # z^T (+ denominator row) raw; host divides and transposes
                    nc.sync.dma_start(
                        out_d[b, h, :, ds(q4 * QT, QT)],
                        zt_sb[h][:, ds(q4 * QT, QT)],
                    )


def build_bass():
    nc = bacc.Bacc(None, target_bir_lowering=False)
    xT_d = nc.declare_dram_parameter(
        "xT", [B, D, T], F32R if PROJ_F32R else BF16, isOutput=False
    )
    w_d = {
        name: nc.declare_dram_parameter(
            name, [P, KD * H2], F32R if PROJ_F32R else BF16, isOutput=False
        )
        for name in ("wq", "wk", "wv")
    }
    cst_d = {}
    out_d = nc.declare_dram_parameter(
        "out", [B, HPC, DH + 1, T], F32, isOutput=True
    )
    with tile.TileContext(nc) as tc:
        _build(nc, tc, xT_d, w_d, cst_d, out_d)
    nc.compile()
    return nc


_CACHE = {}


def _get_nc():
    if "nc" not in _CACHE:
        _CACHE["nc"] = build_bass()
    return _CACHE["nc"]


def make_in_maps(x, W_K, W_Q, W_V):
    x = np.asarray(x, dtype=np.float32)
    xT = np.ascontiguousarray(np.transpose(x, (0, 2, 1)))
    if not PROJ_F32R:
        xT = xT.astype(ml_dtypes.bfloat16)
    in_maps = []
    for c in range(NCORES):
        sl = slice(c * HPC, (c + 1) * HPC)

        def wt(w):
            w = np.asarray(w, dtype=np.float32)
            wt_ = w[sl].reshape(H2, D).T  # [D, H2]
            wt_ = wt_.reshape(KD, P, H2).transpose(1, 0, 2).reshape(P, KD * H2)
            wt_ = np.ascontiguousarray(wt_)
            return wt_ if PROJ_F32R else wt_.astype(ml_dtypes.bfloat16)

        in_maps.append({"xT": xT, "wq": wt(W_Q), "wk": wt(W_K), "wv": wt(W_V)})
    return in_maps


def kernel(x, W_K, W_Q, W_V, _trace=False, _trace_kwargs=None):
    in_maps = make_in_maps(x, W_K, W_Q, W_V)
    res = run_bass_kernel_spmd(
        _get_nc(),
        in_maps,
        list(range(NCORES)),
        trace=_trace,
        **(_trace_kwargs or {}),
    )
    _CACHE["last_results"] = res
    outs = []
    for c in range(NCORES):
        zt = np.asarray(res.results[c]["out"])  # [B, HPC, DH+1, T]
        z = zt[:, :, :DH, :] / zt[:, :, DH : DH + 1, :]
        outs.append(np.transpose(z, (0, 3, 1, 2)).reshape(B, T, H2))
    return np.concatenate(outs, axis=2)


# revision 31
# speedup vs baseline: 1.0655x; 1.0272x over previous
"""Causal multi-head attention (B=4, T=2048, D=1024, 16 heads x 64) on 8 trn2 cores.

Sharding: tensor-parallel over heads, 2 heads per core. Every core receives the
full activations x (pre-transposed on host to [B, D, T], cast bf16) plus its 2
heads' worth of W_Q/W_K/W_V pre-arranged to [128, 8*128] bf16; it computes full
causal attention for its heads and writes out z^T plus the softmax denominator
row per head ([B, 2, 65, T] f32). The host normalizes and re-lays-out.

Device kernel layout choices (per core, HW ~266 us):
  - projections produce Q^T/K^T/V^T head-major [128(2h), T]; V^T is
    PE-transposed per 128-block into the AV stationary (v_aug).
  - scores computed transposed S^T[kt, qt] so the two heads run as concurrent
    PE row-tiles (K=64 at partition bases 0/64, ~4ns apart) into the two
    halves of one [128, 1024] PSUM pair; one Exp (scale=1/8 fused) per chunk.
  - causal mask applied post-exp with gpsimd affine_select (idle engine);
    diagonal chunks narrowed to their valid column range.
  - softmax denominator comes free from an all-ones column in v_aug (padded
    to 128 columns for fast weight load), accumulated in the same f32 PSUM
    as z^T; AV is software-pipelined one chunk behind the scores.
  - all matmuls bf16 with fp32 PSUM accumulation (end-to-end rel err ~5e-3);
    set PROJ_F32R=True for fp32r projections (~3.5e-3, ~13% slower).
"""

import os
import sys

for _p in ("/opt/trn_rl_repo", "/root/.axon_site/_ro/trn_rl_repo"):
    if os.path.isdir(_p) and _p not in sys.path:
        sys.path.insert(0, _p)

import ml_dtypes
import numpy as np

import concourse.bass as bass
import concourse.mybir as mybir
import concourse.tile as tile
from concourse import bacc
from concourse.bass import ds
from concourse.bass_utils import run_bass_kernel_spmd
from concourse.masks import make_identity

B, T, D = 4, 2048, 1024
NH, DH = 16, 64
NCORES = 8
HPC = NH // NCORES          # heads per core = 2
H2 = HPC * DH               # packed per-core head dim = 128
P = 128
QT = 512                    # query-tile width (psum bank limit for f32 out)
NQ4 = T // QT               # 4 query tiles
NCH = T // P                # 16 key chunks
KD = D // P                 # 8 contraction chunks
F32 = mybir.dt.float32
BF16 = mybir.dt.bfloat16
F32R = mybir.dt.float32r
PROJ_F32R = False  # fp32r projections: better precision, slightly slower
SCALE = 1.0 / np.sqrt(DH)   # 0.125


def _build(nc, tc, xT_d, w_d, cst_d, out_d):
    from contextlib import ExitStack

    AF = mybir.ActivationFunctionType
    OP = mybir.AluOpType
    MPB = QT // P  # 128-blocks per query tile = 4

    with ExitStack() as ctx:
        ep = ctx.enter_context
        const = ep(tc.tile_pool(name="const", bufs=1))
        xt_pool = ep(tc.tile_pool(name="xt", bufs=2 * KD + 1))
        qk_pool = ep(tc.tile_pool(name="qk", bufs=2))
        vt_pool = ep(tc.tile_pool(name="vt", bufs=3))
        vaug_pool = ep(tc.tile_pool(name="vaug", bufs=2))
        p_pool = ep(tc.tile_pool(name="pp", bufs=10))
        zt_pool = ep(tc.tile_pool(name="zt", bufs=2))
        ps_acc = ep(tc.tile_pool(name="ps_acc", bufs=2, space="PSUM"))
        ps_s = ep(tc.tile_pool(name="ps_s", bufs=2, space="PSUM"))
        ps_z = ep(tc.tile_pool(name="ps_z", bufs=2, space="PSUM"))

        # Startup: batch 0's x arrives as [128, 512] pieces in t4-major order
        # so the first projection group is never starved waiting on a full
        # 512 KB chunk; weights queue right after the very first piece.
        XDT = F32R if PROJ_F32R else BF16
        xts_pool = ep(tc.tile_pool(name="xts", bufs=NQ4 * KD))
        xch0p = [[None] * KD for _ in range(NQ4)]
        xch0p[0][0] = xts_pool.tile([P, QT], XDT, tag="xts", name="xts")
        nc.sync.dma_start(xch0p[0][0][:], xT_d[0, ds(0, P), ds(0, QT)])
        w_sb = {}
        for name in ("wq", "wk", "wv"):
            t = const.tile([P, KD, H2], XDT, tag=name)
            nc.sync.dma_start(t[:], w_d[name].rearrange("p (c h) -> p c h", c=KD))
            w_sb[name] = t
        for t4 in range(NQ4):
            for k in range(KD):
                if t4 == 0 and k == 0:
                    continue
                tt = xts_pool.tile([P, QT], XDT, tag="xts", name="xts")
                nc.sync.dma_start(tt[:], xT_d[0, ds(k * P, P), ds(t4 * QT, QT)])
                xch0p[t4][k] = tt

        ident = const.tile([P, P], BF16, tag="ident")
        make_identity(nc, ident)


        # v_aug double buffers: [kt, chunk, 64 v-cols | ones col | 63 zeros]
        # (padded to 128 columns so the AV weight load can use FWL)
        vaug = []
        for _bb in range(2):
            pair = []
            for h in range(HPC):
                v = vaug_pool.tile([P, NCH, P], BF16, tag=f"v{h}")
                nc.gpsimd.memset(v[:, :, DH:P], 0.0)
                nc.gpsimd.memset(v[:, :, DH : DH + 1], 1.0)
                pair.append(v)
            vaug.append(pair)

        for b in range(B):
            # ---- stream x^T chunks [128d, T] ----
            if b == 0:
                xch = None  # batch 0 uses the prefetched per-t4 pieces
            else:
                xch = []
                for k in range(KD):
                    xt_t = xt_pool.tile([P, T], XDT, tag="xt", name="xt_t")
                    nc.sync.dma_start(xt_t[:], xT_d[b, ds(k * P, P), :])
                    xch.append(xt_t)

            # ---- projections: Q^T, K^T (kept), V^T (transposed to v_aug) ----
            qt_sb = qk_pool.tile([P, T], BF16, tag="qt")
            kt_sb = qk_pool.tile([P, T], BF16, tag="kt")
            va = vaug[b % 2]
            for t4 in range(NQ4):
                for name, dst in (("wq", qt_sb), ("wk", kt_sb), ("wv", None)):
                    acc = ps_acc.tile([P, QT], F32, tag="acc")
                    for k in range(KD):
                        rhs = (
                            xch0p[t4][k][:]
                            if b == 0
                            else xch[k][:, ds(t4 * QT, QT)]
                        )
                        nc.tensor.matmul(
                            acc[:],
                            w_sb[name][:, k, :],
                            rhs,
                            start=(k == 0),
                            stop=(k == KD - 1),
                        )
                    if dst is not None:
                        nc.vector.tensor_copy(dst[:, ds(t4 * QT, QT)], acc[:])
                    else:
                        vt_t = vt_pool.tile([P, QT], BF16, tag="vt")
                        nc.vector.tensor_copy(vt_t[:], acc[:])
                        for m in range(MPB):
                            j = t4 * MPB + m
                            pt = ps_acc.tile([P, P], BF16, tag="acc", name="pt")
                            nc.tensor.transpose(
                                pt[:], vt_t[:, ds(m * P, P)], ident[:]
                            )
                            for h in range(HPC):
                                nc.vector.tensor_copy(
                                    va[h][:, j, 0:DH], pt[:, ds(h * DH, DH)]
                                )

            # ---- causal attention; scores for both heads side by side ----
            zt_sb = [
                zt_pool.tile([DH + 1, T], F32, tag=f"z{h}", name=f"ztb{h}")
                for h in range(HPC)
            ]
            for q4 in range(NQ4):
                njs = (q4 + 1) * MPB
                pz = [
                    ps_z.tile([P, QT], F32, tag="z", name="pz") for _ in range(HPC)
                ]
                pend = []  # (j, c0, exp tile) awaiting the AV matmuls
                for j in range(njs):
                    rdiag = j - q4 * MPB  # >=0 on diagonal-overlap chunks
                    last = j == njs - 1
                    c0 = 0 if rdiag < 0 else rdiag * P
                    w_hi = (rdiag + 1) * P if rdiag >= 0 else 0
                    nw = QT - c0
                    ss = ps_s.tile([P, 2 * QT], F32, tag="s")
                    pe = p_pool.tile([P, 2 * QT], BF16, tag="p", name="pe")
                    for h in range(HPC):
                        hp = ds(h * DH, DH)
                        nc.tensor.matmul(
                            ss[:, h * QT + c0 : (h + 1) * QT],
                            kt_sb[hp, ds(j * P, P)],
                            qt_sb[hp, ds(q4 * QT + c0, nw)],
                            start=True,
                            stop=True,
                        )
                    # one exp covering both heads' valid halves
                    if c0 == 0:
                        nc.scalar.activation(
                            pe[:, :], ss[:, :], AF.Exp, scale=float(SCALE)
                        )
                    else:
                        for h in range(HPC):
                            nc.scalar.activation(
                                pe[:, h * QT + c0 : (h + 1) * QT],
                                ss[:, h * QT + c0 : (h + 1) * QT],
                                AF.Exp,
                                scale=float(SCALE),
                            )
                    if rdiag >= 0:
                        # keep iff qt >= kt  <=>  (col - p - 128*rdiag) >= 0
                        for h in range(HPC):
                            nc.gpsimd.affine_select(
                                out=pe[:, h * QT + c0 : h * QT + w_hi],
                                in_=pe[:, h * QT + c0 : h * QT + w_hi],
                                compare_op=OP.is_ge,
                                fill=0.0,
                                base=c0 - rdiag * P,
                                pattern=[[1, w_hi - c0]],
                                channel_multiplier=-1,
                            )
                    pend.append((j, c0, pe))
                    # software-pipeline: AV runs one chunk behind the scores
                    if len(pend) > 3 or last:
                        for jj, cc0, ppe in pend if last else [pend[0]]:
                            for h in range(HPC):
                                nc.tensor.matmul(
                                    pz[h][:, cc0:QT],
                                    va[h][:, jj, :],
                                    ppe[:, h * QT + cc0 : (h + 1) * QT],
                                    start=(jj == 0),
                                    stop=(jj == njs - 1),
                                    skip_group_check=True,
                                )
                        pend = [] if last else pend[1:]

                for h in range(HPC):
                    nc.vector.tensor_copy(
                        zt_sb[h][:, ds(q4 * QT, QT)], pz[h][0 : DH + 1, :]
                    )
                    # z^T (+ denominator row) raw; host divides and transposes
                    nc.sync.dma_start(
                        out_d[b, h, :, ds(q4 * QT, QT)],
                        zt_sb[h][:, ds(q4 * QT, QT)],
                    )


def build_bass():
    nc = bacc.Bacc(None, target_bir_lowering=False)
    xT_d = nc.declare_dram_parameter(
        "xT", [B, D, T], F32R if PROJ_F32R else BF16, isOutput=False
    )
    w_d = {
        name: nc.declare_dram_parameter(
            name, [P, KD * H2], F32R if PROJ_F32R else BF16, isOutput=False
        )
        for name in ("wq", "wk", "wv")
    }
    cst_d = {}
    out_d = nc.declare_dram_parameter(
        "out", [B, HPC, DH + 1, T], F32, isOutput=True
    )
    with tile.TileContext(nc) as tc:
        _build(nc, tc, xT_d, w_d, cst_d, out_d)
    nc.compile()
    return nc


_CACHE = {}


def _get_nc():
    if "nc" not in _CACHE:
        _CACHE["nc"] = build_bass()
    return _CACHE["nc"]


def make_in_maps(x, W_K, W_Q, W_V):
    x = np.asarray(x, dtype=np.float32)
    xT = np.ascontiguousarray(np.transpose(x, (0, 2, 1)))
    if not PROJ_F32R:
        xT = xT.astype(ml_dtypes.bfloat16)
    in_maps = []
    for c in range(NCORES):
        sl = slice(c * HPC, (c + 1) * HPC)

        def wt(w):
            w = np.asarray(w, dtype=np.float32)
            wt_ = w[sl].reshape(H2, D).T  # [D, H2]
            wt_ = wt_.reshape(KD, P, H2).transpose(1, 0, 2).reshape(P, KD * H2)
            wt_ = np.ascontiguousarray(wt_)
            return wt_ if PROJ_F32R else wt_.astype(ml_dtypes.bfloat16)

        in_maps.append({"xT": xT, "wq": wt(W_Q), "wk": wt(W_K), "wv": wt(W_V)})
    return in_maps


def kernel(x, W_K, W_Q, W_V, _trace=False, _trace_kwargs=None):
    in_maps = make_in_maps(x, W_K, W_Q, W_V)
    res = run_bass_kernel_spmd(
        _get_nc(),
        in_maps,
        list(range(NCORES)),
        trace=_trace,
        **(_trace_kwargs or {}),
    )
    _CACHE["last_results"] = res
    outs = []
    for c in range(NCORES):
        zt = np.asarray(res.results[c]["out"])  # [B, HPC, DH+1, T]
        z = zt[:, :, :DH, :] / zt[:, :, DH : DH + 1, :]
        outs.append(np.transpose(z, (0, 3, 1, 2)).reshape(B, T, H2))
    return np.concatenate(outs, axis=2)
